# revision 2
# baseline (speedup 1.0000x reference)
"""Trainium2 Bass kernel for nn_Capsule (capsule attention w/ dynamic routing).

Math: in the reference, c = softmax(b, axis=1) is over a size-1 axis, so c == 1
for every routing iteration. The whole module therefore collapses to

    s[b, d] = sum_{j,e} W[0, j, d, e] * x[b, j, e]        (one big matmul)
    out     = squash(s)  -> (B, 1, D)

i.e. a (512, 36*1024) @ (36*1024, 1024) matmul followed by a per-row squash.

Sharding: contraction(K)-parallel over 8 cores. Each core gets K/8 = 4608 rows
of x^T and W^T (laid out host-side in SBUF-friendly [128, kt*free] order, cast
to bf16) and computes a partial (512, 1024) fp32 sum on its TensorEngine.
The host unshard step sums the 8 partials and applies squash.

K-sharding moves ~14 MB/core from HBM vs ~151 MB/core for the data-parallel
(replicated-weight) alternative; the kernel runs at the bf16 TensorE roofline.
"""

import os
import sys

for _p in ("/opt/trn_rl_repo", "/root/.axon_site/_ro/trn_rl_repo"):
    if os.path.isdir(_p) and _p not in sys.path:
        sys.path.append(_p)

import ml_dtypes
import numpy as np

N_CAPS = 36
D = 1024
B = 512
N_CORES = 8
K = N_CAPS * D            # 36864 contraction length
KC = K // N_CORES         # 4608 per core
KT = KC // 128            # 36 k-tiles of 128 per core
B_TILES = B // 128        # 4
D_CHUNKS = D // 512       # 2

_CACHE = {}
LAST_RESULTS = None       # BassKernelResults of the most recent run (for profiling)


def _build():
    import concourse.mybir as mybir
    import concourse.tile as tile
    from concourse import bacc

    nc = bacc.Bacc("TRN2", target_bir_lowering=False, debug=False,
                   num_devices=N_CORES)

    # Per-core inputs, already transposed/tiled host-side:
    #   xt[p, kt*512 + b] = x[b, k0 + kt*128 + p]   (k = j*1024+e flattened)
    #   wt[p, kt*1024 + d] = W[0, j, d, e] at k = k0 + kt*128 + p
    xt = nc.dram_tensor("xt", [128, KT * B], mybir.dt.bfloat16, kind="ExternalInput")
    wt = nc.dram_tensor("wt", [128, KT * D], mybir.dt.bfloat16, kind="ExternalInput")
    out = nc.dram_tensor("out", [B, D], mybir.dt.float32, kind="ExternalOutput")

    with tile.TileContext(nc) as tc:
        with tc.tile_pool(name="xpool", bufs=1) as xpool, \
             tc.tile_pool(name="wpool", bufs=1) as wpool, \
             tc.tile_pool(name="warm", bufs=1) as warm_pool, \
             tc.tile_pool(name="stage", bufs=8) as stage_pool, \
             tc.tile_pool(name="psum", bufs=8, space="PSUM") as psum_pool:

            X = xpool.tile([128, KT * B], mybir.dt.bfloat16, name="X")
            W = wpool.tile([128, KT * D], mybir.dt.bfloat16, name="W")

            # PE clock warmup: HAM un-throttles (1.2 -> 2.4 GHz) after
            # ~3.4us of sustained matmul activity. Run dummy matmuls on a
            # scratch tile while the first DMA chunks are in flight so the
            # real matmuls start at full clock.
            ws = warm_pool.tile([128, 128], mybir.dt.bfloat16, name="ws")
            wp = psum_pool.tile([128, 512], mybir.dt.float32, name="wp", tag="ps")
            nc.gpsimd.memset(ws[:, :], 0.0)
            for i in range(40):
                nc.tensor.matmul(wp[:, 0:128], lhsT=ws[:, :], rhs=ws[:, :],
                                 start=True, stop=True)

            # Stream inputs in ramped kt-chunks (small first so matmuls
            # start early, large later for DMA efficiency). W chunks on the
            # Sync HWDGE ring, X chunks on the Scalar ring (parallel issue).
            CHUNKS = [1, 1, 2, 4, 4, 6, 6, 6, 6]
            s = 0
            for ch in CHUNKS:
                nc.sync.dma_start(out=W[:, s * D:(s + ch) * D],
                                  in_=wt[:, s * D:(s + ch) * D])
                nc.scalar.dma_start(out=X[:, s * B:(s + ch) * B],
                                    in_=xt[:, s * B:(s + ch) * B])
                s += ch

            # Two phases (d-chunk 0 then 1); 4 psum banks accumulate per
            # phase, so phase-0 PSUM->SBUF->DRAM drains overlap phase-1
            # matmuls and the kernel tail stays short.
            for d in range(D_CHUNKS):
                psums = []
                for b in range(B_TILES):
                    pt = psum_pool.tile([128, 512], mybir.dt.float32,
                                        name=f"ps_{d}_{b}", tag="ps")
                    psums.append(pt)
                for kt in range(KT):
                    for b in range(B_TILES):
                        nc.tensor.matmul(
                            psums[b][:, :],
                            lhsT=X[:, kt * B + b * 128: kt * B + (b + 1) * 128],
                            rhs=W[:, kt * D + d * 512: kt * D + (d + 1) * 512],
                            start=(kt == 0),
                            stop=(kt == KT - 1),
                        )
                for b in range(B_TILES):
                    st = stage_pool.tile([128, 512], mybir.dt.float32,
                                         name=f"st_{d}_{b}", tag="st")
                    nc.vector.tensor_copy(st[:, :], psums[b][:, :])
                    nc.sync.dma_start(
                        out=out[b * 128:(b + 1) * 128, d * 512:(d + 1) * 512],
                        in_=st[:, :])

    nc.compile()
    return nc


def _get_nc():
    if "nc" not in _CACHE:
        _CACHE["nc"] = _build()
    return _CACHE["nc"]


def _shard_inputs(x, weight):
    """Host-side layout prep: transpose to k-major, tile for SBUF, cast bf16."""
    bf16 = ml_dtypes.bfloat16
    # x: (B, 36, 1024) -> xT (K, B) -> per-core [128, KT*B]
    xT = np.ascontiguousarray(np.transpose(x, (1, 2, 0))).reshape(K, B)
    xts = (xT.reshape(N_CORES, KT, 128, B)
              .transpose(0, 2, 1, 3)
              .reshape(N_CORES, 128, KT * B)
              .astype(bf16))
    # weight: (1, 36, D, E) -> Wk (K, D) with k=(j,e) -> per-core [128, KT*D]
    wk = np.ascontiguousarray(np.transpose(weight[0], (0, 2, 1))).reshape(K, D)
    wts = (wk.reshape(N_CORES, KT, 128, D)
              .transpose(0, 2, 1, 3)
              .reshape(N_CORES, 128, KT * D)
              .astype(bf16))
    return xts, wts


def kernel(x, weight, isLastLayer=None):
    global LAST_RESULTS
    from concourse.bass_utils import run_bass_kernel_spmd

    x = np.asarray(x, dtype=np.float32)
    weight = np.asarray(weight, dtype=np.float32)
    assert x.shape == (B, N_CAPS, D) and weight.shape == (1, N_CAPS, D, D)

    xts, wts = _shard_inputs(x, weight)
    in_maps = [{"xt": np.ascontiguousarray(xts[i]),
                "wt": np.ascontiguousarray(wts[i])} for i in range(N_CORES)]

    nc = _get_nc()
    res = run_bass_kernel_spmd(nc, in_maps, core_ids=list(range(N_CORES)))
    LAST_RESULTS = res

    # Unshard: sum the 8 contraction partials, then squash.
    s = np.zeros((B, D), dtype=np.float32)
    for core_out in res.results:
        s += np.asarray(core_out["out"], dtype=np.float32)
    norm = np.sqrt((s.astype(np.float64) ** 2).sum(axis=-1, keepdims=True)).astype(np.float32)
    scale = norm ** 2 / (1.0 + norm ** 2) / (norm + 1e-8)
    return (scale * s)[:, None, :].astype(np.float32)


# revision 7
# speedup vs baseline: 1.1352x; 1.1352x over previous
"""Trainium2 Bass kernel for nn_Capsule (capsule attention w/ dynamic routing).

Math: in the reference, c = softmax(b, axis=1) is over a size-1 axis, so
c == 1 in every routing iteration and the module collapses to

    s[b, d] = sum_{j,e} W[0, j, d, e] * x[b, j, e]     (one big matmul)
    out     = squash(s)                                 -> (B, 1, D)

i.e. (512, 36*1024) @ (36*1024, 1024) followed by a per-row squash.

Sharding: contraction(K)-parallel over 8 NeuronCores. Each core gets
K/8 = 4608 rows of x^T and W^T (host-side layout: k-major, SBUF-tiled
[128, kt*free], bf16) and computes a partial (512, 1024) sum at the bf16
TensorEngine roofline (~61.5us of matmul). The host unshard step sums the
8 partials and applies squash. K-sharding moves ~14 MB/core from HBM vs
~151 MB/core for data-parallel (replicated weight).

Hand-scheduled raw Bass (no Tile): single interleaved pass where all 8
PSUM banks (4 b-tiles x 2 d-chunks) accumulate per k-tile, so each DMA
chunk is consumed once and the PE is the only steady-state bottleneck.

Engine plan:
  SP  (sync):   W chunk DMAs (HWDGE), out DMAs b0/b1, final wait + cleanup
  ACT (scalar): X chunk DMAs (HWDGE ring #2), out DMAs b2/b3 (no activation
                ops on ACT -> no ACT table load in the startup path)
  PE  (tensor): warmup matmuls (HAM clock ramp), then 288 real matmuls;
                last 4 k-tiles run bank-major so the copy/DMA tail hides
                behind the matmul stream
  DVE (vector): PSUM -> SBUF staging copies (fp32 -> bf16 cast)
"""

import os
import sys
from contextlib import ExitStack

for _p in ("/opt/trn_rl_repo", "/root/.axon_site/_ro/trn_rl_repo"):
    if os.path.isdir(_p) and _p not in sys.path:
        sys.path.append(_p)

import ml_dtypes
import numpy as np

N_CAPS = 36
D = 1024
B = 512
N_CORES = 8
K = N_CAPS * D
KC = K // N_CORES
KT = KC // 128            # 36
B_TILES = B // 128        # 4
D_CHUNKS = D // 512       # 2
CHUNKS = [1, 1, 2, 2, 3, 3, 6, 6, 6, 6]   # kt per DMA chunk (ramped)
N_WARM = 18

_CACHE = {}
LAST_RESULTS = None


def _build():
    import concourse.bass as bass
    import concourse.mybir as mybir
    from concourse import bacc

    nc = bacc.Bacc("TRN2", target_bir_lowering=False, debug=False,
                   num_devices=N_CORES)
    bf16 = mybir.dt.bfloat16
    f32 = mybir.dt.float32

    xt = nc.dram_tensor("xt", [128, KT * B], bf16, kind="ExternalInput")
    wt = nc.dram_tensor("wt", [128, KT * D], bf16, kind="ExternalInput")
    out = nc.dram_tensor("out", [B, D], bf16, kind="ExternalOutput")

    bounds = []
    s = 0
    for ch in CHUNKS:
        bounds.append((s, ch))
        s += ch
    assert s == KT

    with ExitStack() as ctx:
        X = ctx.enter_context(nc.sbuf_tensor("X", [128, KT * B], bf16))
        W = ctx.enter_context(nc.sbuf_tensor("W", [128, KT * D], bf16))
        scratch = ctx.enter_context(nc.sbuf_tensor("scratch", [128, 256], bf16))
        stagings = [
            ctx.enter_context(nc.sbuf_tensor(f"st{b}", [128, 1024], bf16))
            for b in range(B_TILES)
        ]
        psums = [
            ctx.enter_context(nc.psum_tensor(f"ps{g}", [128, 512], f32))
            for g in range(8)
        ]
        # One completion sem per chunk DMA: a single shared counting sem is
        # NOT safe across dma_starts (each DMA's 16 per-SDMA-engine
        # increments interleave with the next DMA's, so a >=16*k threshold
        # can fire before chunk k-1 fully lands).
        w_sems = [ctx.enter_context(nc.semaphore(f"w_sem{i}"))
                  for i in range(len(CHUNKS))]
        x_sems = [ctx.enter_context(nc.semaphore(f"x_sem{i}"))
                  for i in range(len(CHUNKS))]
        pe_sem = ctx.enter_context(nc.semaphore("pe_sem"))
        cp_sem = ctx.enter_context(nc.semaphore("cp_sem"))
        out_sem = ctx.enter_context(nc.semaphore("out_sem"))
        all_sems = w_sems + x_sems + [pe_sem, cp_sem, out_sem]

        with nc.Block(no_gpsimd_drain=True) as block:

            @block.sync
            def _(sync):
                for ci, (s0, ch) in enumerate(bounds):
                    sync.dma_start(
                        out=W[:, s0 * D:(s0 + ch) * D],
                        in_=wt[:, s0 * D:(s0 + ch) * D],
                    ).then_inc(w_sems[ci], 16)
                for b in (0, 1):
                    sync.wait_ge(cp_sem, 2 * (b + 1))
                    sync.dma_start(
                        out=out[b * 128:(b + 1) * 128, :],
                        in_=stagings[b][:, :],
                    ).then_inc(out_sem, 16)
                sync.wait_ge(out_sem, 16 * 4)
                # zero sems so the NEFF can re-execute (out_sem>=64 proves
                # every DMA completed; no in-flight increments remain)
                nums = sorted(sm.num for sm in all_sems)
                lo, hi = nums[0], nums[-1]
                if nums == list(range(lo, hi + 1)):
                    sync.sem_clear(range(lo, hi + 1))
                else:
                    for sm in all_sems:
                        sync.sem_clear(range(sm.num, sm.num + 1))

            @block.scalar
            def _(scalar):
                for ci, (s0, ch) in enumerate(bounds):
                    scalar.dma_start(
                        out=X[:, s0 * B:(s0 + ch) * B],
                        in_=xt[:, s0 * B:(s0 + ch) * B],
                    ).then_inc(x_sems[ci], 16)
                # out DMAs for b2/b3 on the ACT HWDGE ring (copies stay on
                # DVE: ACT's activation-path copy is not bit-exact).
                for b in (2, 3):
                    scalar.wait_ge(cp_sem, 2 * (b + 1))
                    scalar.dma_start(
                        out=out[b * 128:(b + 1) * 128, :],
                        in_=stagings[b][:, :],
                    ).then_inc(out_sem, 16)

            @block.tensor
            def _(tensor):
                # warm the PE clock while DMAs stream (results discarded)
                for _i in range(N_WARM):
                    tensor.matmul(psums[7][:, 0:256], lhsT=scratch[:, 0:128],
                                  rhs=scratch[:, :], start=True, stop=True)
                def mm_for(kt, b, dd):
                    g = b * 2 + dd
                    mm = tensor.matmul(
                        psums[g][:, :],
                        lhsT=X[:, kt * B + b * 128: kt * B + (b + 1) * 128],
                        rhs=W[:, kt * D + dd * 512: kt * D + (dd + 1) * 512],
                        start=(kt == 0),
                        stop=(kt == KT - 1),
                    )
                    if kt == KT - 1:
                        mm.then_inc(pe_sem, 1)

                # kt-major over kt 0..KT-5 (tracks DMA chunk arrival), then
                # bank-major for the last 4 k-tiles so early banks finish
                # ~7us before the stream ends and the DVE copy chain +
                # out-DMA receipts hide behind the matmul tail.
                TAIL_KT = 4
                chunk_idx = 0
                next_boundary = 0
                for kt in range(KT - TAIL_KT):
                    if kt == next_boundary:
                        tensor.wait_ge(w_sems[chunk_idx], 16)
                        tensor.wait_ge(x_sems[chunk_idx], 16)
                        next_boundary += CHUNKS[chunk_idx]
                        chunk_idx += 1
                    for b in range(B_TILES):
                        for dd in range(D_CHUNKS):
                            mm_for(kt, b, dd)
                while chunk_idx < len(CHUNKS):
                    tensor.wait_ge(w_sems[chunk_idx], 16)
                    tensor.wait_ge(x_sems[chunk_idx], 16)
                    chunk_idx += 1
                for b in range(B_TILES):
                    for dd in range(D_CHUNKS):
                        for kt in range(KT - TAIL_KT, KT):
                            mm_for(kt, b, dd)

            @block.vector
            def _(vector):
                for g in range(8):
                    b, dd = divmod(g, 2)
                    vector.wait_ge(pe_sem, g + 1)
                    vector.tensor_copy(
                        stagings[b][:, dd * 512:(dd + 1) * 512],
                        psums[g][:, :],
                    ).then_inc(cp_sem, 1)

    nc.compile()
    return nc


def _get_nc():
    if "nc" not in _CACHE:
        _CACHE["nc"] = _build()
    return _CACHE["nc"]


def _shard_inputs(x, weight):
    bf16 = ml_dtypes.bfloat16
    xT = np.ascontiguousarray(np.transpose(x, (1, 2, 0))).reshape(K, B)
    xts = (xT.reshape(N_CORES, KT, 128, B)
              .transpose(0, 2, 1, 3)
              .reshape(N_CORES, 128, KT * B)
              .astype(bf16))
    wk = np.ascontiguousarray(np.transpose(weight[0], (0, 2, 1))).reshape(K, D)
    wts = (wk.reshape(N_CORES, KT, 128, D)
              .transpose(0, 2, 1, 3)
              .reshape(N_CORES, 128, KT * D)
              .astype(bf16))
    return xts, wts


def _ensure_trace_shim():
    """If the environment requests NTFF tracing (BASS_TRACE=1) but this
    container's antenv lacks axon_hooks, provide it from trn_boot's ctypes
    implementation so run_bass_kernel_spmd doesn't crash mid-trace."""
    try:
        import antenv.axon_hooks  # noqa: F401
        return
    except ImportError:
        pass
    try:
        import types

        import antenv
        import trn_agent_boot.trn_boot as tb
        from concourse import bass_utils

        hook = tb._ntff_profile_via_ctypes("/opt/axon/libaxon_pjrt.so")
        mod = types.ModuleType("antenv.axon_hooks")
        mod.get_axon_ntff_profile_hook = lambda: hook
        mod.set_axon_ntff_profile_hook = lambda h: None
        antenv.axon_hooks = mod
        sys.modules["antenv.axon_hooks"] = mod
        if not getattr(bass_utils.upload_artifacts, "_patched", False):
            bass_utils.upload_artifacts = lambda tmpdir: tmpdir
            bass_utils.upload_artifacts._patched = True
    except Exception:
        # tracing unavailable -> disable rather than crash the run
        os.environ["BASS_NEVER_TRACE"] = "1"


def kernel(x, weight, isLastLayer=None):
    global LAST_RESULTS
    _ensure_trace_shim()
    from concourse.bass_utils import run_bass_kernel_spmd

    x = np.asarray(x, dtype=np.float32)
    weight = np.asarray(weight, dtype=np.float32)

    xts, wts = _shard_inputs(x, weight)
    in_maps = [{"xt": np.ascontiguousarray(xts[i]),
                "wt": np.ascontiguousarray(wts[i])} for i in range(N_CORES)]

    nc = _get_nc()
    res = run_bass_kernel_spmd(nc, in_maps, core_ids=list(range(N_CORES)))
    LAST_RESULTS = res

    s = np.zeros((B, D), dtype=np.float32)
    for core_out in res.results:
        s += np.asarray(core_out["out"]).astype(np.float32)
    norm = np.sqrt((s.astype(np.float64) ** 2).sum(axis=-1, keepdims=True)).astype(np.float32)
    scale = norm ** 2 / (1.0 + norm ** 2) / (norm + 1e-8)
    return (scale * s)[:, None, :].astype(np.float32)


# revision 13
# speedup vs baseline: 1.1579x; 1.0200x over previous
"""Trainium2 Bass kernel for nn_Capsule (capsule attention w/ dynamic routing).

Math: in the reference, c = softmax(b, axis=1) is over a size-1 axis, so
c == 1 in every routing iteration and the module collapses to

    s[b, d] = sum_{j,e} W[0, j, d, e] * x[b, j, e]     (one big matmul)
    out     = squash(s)                                 -> (B, 1, D)

i.e. (512, 36*1024) @ (36*1024, 1024) followed by a per-row squash.

Sharding: contraction(K)-parallel over 8 NeuronCores. Each core gets
K/8 = 4608 rows of x^T and W^T (host-side layout: k-major, SBUF-tiled
[128, kt*free], bf16) and computes a partial (512, 1024) sum at the bf16
TensorEngine roofline (~61.5us of matmul). The host unshard step sums the
8 partials and applies squash. K-sharding moves ~14 MB/core from HBM vs
~151 MB/core for data-parallel (replicated weight).

Hand-scheduled raw Bass (no Tile): single interleaved pass where all 8
PSUM banks (4 b-tiles x 2 d-chunks) accumulate per k-tile, so each DMA
chunk is consumed once and the PE is the only steady-state bottleneck.

Engine plan:
  SP  (sync):   W chunk DMAs (HWDGE), out DMAs b0/b1, final wait + cleanup
  ACT (scalar): X chunk DMAs (HWDGE ring #2), out DMAs b2/b3 (no activation
                ops on ACT -> no ACT table load in the startup path)
  PE  (tensor): warmup matmuls (HAM clock ramp), then 288 real matmuls;
                last 4 k-tiles run bank-major so the copy/DMA tail hides
                behind the matmul stream
  DVE (vector): PSUM -> SBUF staging copies (fp32 -> bf16 cast)
"""

import os
import sys
from contextlib import ExitStack

for _p in ("/opt/trn_rl_repo", "/root/.axon_site/_ro/trn_rl_repo"):
    if os.path.isdir(_p) and _p not in sys.path:
        sys.path.append(_p)

import ml_dtypes
import numpy as np

N_CAPS = 36
D = 1024
B = 512
N_CORES = 8
K = N_CAPS * D
KC = K // N_CORES
KT = KC // 128            # 36
B_TILES = B // 128        # 4
D_CHUNKS = D // 512       # 2
CHUNKS = [1, 1, 2, 2, 3, 3, 6, 6, 6, 6]   # kt per DMA chunk (ramped)
N_WARM = 18

_CACHE = {}
LAST_RESULTS = None


def _build():
    import concourse.bass as bass
    import concourse.mybir as mybir
    from concourse import bacc

    nc = bacc.Bacc("TRN2", target_bir_lowering=False, debug=False,
                   num_devices=N_CORES)
    bf16 = mybir.dt.bfloat16
    f32 = mybir.dt.float32

    # Inputs are stored chunk-major (each DMA chunk is one fully contiguous
    # HBM block) so early chunks stream at full sequential bandwidth.
    xt = nc.dram_tensor("xt", [128 * KT * B], bf16, kind="ExternalInput")
    wt = nc.dram_tensor("wt", [128 * KT * D], bf16, kind="ExternalInput")
    out = nc.dram_tensor("out", [B, D], bf16, kind="ExternalOutput")

    bounds = []
    s = 0
    for ch in CHUNKS:
        bounds.append((s, ch))
        s += ch
    assert s == KT

    with ExitStack() as ctx:
        X = ctx.enter_context(nc.sbuf_tensor("X", [128, KT * B], bf16))
        W = ctx.enter_context(nc.sbuf_tensor("W", [128, KT * D], bf16))
        scratch = ctx.enter_context(nc.sbuf_tensor("scratch", [128, 256], bf16))
        stagings = [
            ctx.enter_context(nc.sbuf_tensor(f"st{b}", [128, 1024], bf16))
            for b in range(B_TILES)
        ]
        psums = [
            ctx.enter_context(nc.psum_tensor(f"ps{g}", [128, 512], f32))
            for g in range(8)
        ]
        # One completion sem per chunk DMA: a single shared counting sem is
        # NOT safe across dma_starts (each DMA's 16 per-SDMA-engine
        # increments interleave with the next DMA's, so a >=16*k threshold
        # can fire before chunk k-1 fully lands).
        w_sems = [ctx.enter_context(nc.semaphore(f"w_sem{i}"))
                  for i in range(len(CHUNKS))]
        x_sems = [ctx.enter_context(nc.semaphore(f"x_sem{i}"))
                  for i in range(len(CHUNKS))]
        pe_sem = ctx.enter_context(nc.semaphore("pe_sem"))
        cp_sem = ctx.enter_context(nc.semaphore("cp_sem"))
        out_sem = ctx.enter_context(nc.semaphore("out_sem"))
        all_sems = w_sems + x_sems + [pe_sem, cp_sem, out_sem]

        with nc.Block(no_gpsimd_drain=True) as block:

            @block.sync
            def _(sync):
                for ci, (s0, ch) in enumerate(bounds):
                    src = wt[128 * s0 * D: 128 * (s0 + ch) * D] \
                        .rearrange("(p f) -> p f", p=128)
                    sync.dma_start(
                        out=W[:, s0 * D:(s0 + ch) * D],
                        in_=src,
                    ).then_inc(w_sems[ci], 16)
                for b in (0, 1):
                    sync.wait_ge(cp_sem, 2 * (b + 1))
                    sync.dma_start(
                        out=out[b * 128:(b + 1) * 128, :],
                        in_=stagings[b][:, :],
                    ).then_inc(out_sem, 16)
                sync.wait_ge(out_sem, 16 * 5)
                # zero sems so the NEFF can re-execute (out_sem>=64 proves
                # every DMA completed; no in-flight increments remain)
                nums = sorted(sm.num for sm in all_sems)
                lo, hi = nums[0], nums[-1]
                if nums == list(range(lo, hi + 1)):
                    sync.sem_clear(range(lo, hi + 1))
                else:
                    for sm in all_sems:
                        sync.sem_clear(range(sm.num, sm.num + 1))

            @block.scalar
            def _(scalar):
                for ci, (s0, ch) in enumerate(bounds):
                    src = xt[128 * s0 * B: 128 * (s0 + ch) * B] \
                        .rearrange("(p f) -> p f", p=128)
                    scalar.dma_start(
                        out=X[:, s0 * B:(s0 + ch) * B],
                        in_=src,
                    ).then_inc(x_sems[ci], 16)
                # out DMAs for b2/b3 on the ACT HWDGE ring (copies stay on
                # DVE: ACT's activation-path copy is not bit-exact). b3 is
                # the critical tail: ship each half as soon as its copy
                # lands so the g6-half transfer overlaps the g7 copy.
                scalar.wait_ge(cp_sem, 6)
                scalar.dma_start(
                    out=out[2 * 128:3 * 128, :],
                    in_=stagings[2][:, :],
                ).then_inc(out_sem, 16)
                for dd in range(2):
                    scalar.wait_ge(cp_sem, 7 + dd)
                    scalar.dma_start(
                        out=out[3 * 128:4 * 128, dd * 512:(dd + 1) * 512],
                        in_=stagings[3][:, dd * 512:(dd + 1) * 512],
                    ).then_inc(out_sem, 16)

            @block.tensor
            def _(tensor):
                # warm the PE clock while DMAs stream (results discarded)
                for _i in range(N_WARM):
                    tensor.matmul(psums[7][:, 0:256], lhsT=scratch[:, 0:128],
                                  rhs=scratch[:, :], start=True, stop=True)
                def mm_for(kt, b, dd):
                    g = b * 2 + dd
                    mm = tensor.matmul(
                        psums[g][:, :],
                        lhsT=X[:, kt * B + b * 128: kt * B + (b + 1) * 128],
                        rhs=W[:, kt * D + dd * 512: kt * D + (dd + 1) * 512],
                        start=(kt == 0),
                        stop=(kt == KT - 1),
                    )
                    if kt == KT - 1:
                        mm.then_inc(pe_sem, 1)

                # kt-major over kt 0..KT-5 (tracks DMA chunk arrival), then
                # bank-major for the last 4 k-tiles so early banks finish
                # ~7us before the stream ends and the DVE copy chain +
                # out-DMA receipts hide behind the matmul tail.
                TAIL_KT = 4
                chunk_idx = 0
                next_boundary = 0
                for kt in range(KT - TAIL_KT):
                    if kt == next_boundary:
                        tensor.wait_ge(w_sems[chunk_idx], 16)
                        tensor.wait_ge(x_sems[chunk_idx], 16)
                        next_boundary += CHUNKS[chunk_idx]
                        chunk_idx += 1
                    for b in range(B_TILES):
                        for dd in range(D_CHUNKS):
                            mm_for(kt, b, dd)
                while chunk_idx < len(CHUNKS):
                    tensor.wait_ge(w_sems[chunk_idx], 16)
                    tensor.wait_ge(x_sems[chunk_idx], 16)
                    chunk_idx += 1
                for b in range(B_TILES):
                    for dd in range(D_CHUNKS):
                        for kt in range(KT - TAIL_KT, KT):
                            mm_for(kt, b, dd)

            @block.vector
            def _(vector):
                for g in range(8):
                    b, dd = divmod(g, 2)
                    vector.wait_ge(pe_sem, g + 1)
                    vector.tensor_copy(
                        stagings[b][:, dd * 512:(dd + 1) * 512],
                        psums[g][:, :],
                    ).then_inc(cp_sem, 1)

    nc.compile()
    return nc


def _get_nc():
    if "nc" not in _CACHE:
        _CACHE["nc"] = _build()
    return _CACHE["nc"]


def _chunk_major(a, cols):
    """[N_CORES, 128, KT*cols] -> [N_CORES, 128*KT*cols] with each DMA
    chunk's [128, ch*cols] block stored contiguously (kernel reads chunk ci
    at flat offset 128*s0*cols)."""
    n = a.shape[0]
    flat = np.empty((n, 128 * KT * cols), dtype=a.dtype)
    s = 0
    for ch in CHUNKS:
        blk = a[:, :, s * cols:(s + ch) * cols]
        flat[:, 128 * s * cols:128 * (s + ch) * cols] = blk.reshape(n, -1)
        s += ch
    return flat


def _shard_inputs(x, weight):
    bf16 = ml_dtypes.bfloat16
    xT = np.ascontiguousarray(np.transpose(x, (1, 2, 0))).reshape(K, B)
    xts = (xT.reshape(N_CORES, KT, 128, B)
              .transpose(0, 2, 1, 3)
              .reshape(N_CORES, 128, KT * B)
              .astype(bf16))
    wk = np.ascontiguousarray(np.transpose(weight[0], (0, 2, 1))).reshape(K, D)
    wts = (wk.reshape(N_CORES, KT, 128, D)
              .transpose(0, 2, 1, 3)
              .reshape(N_CORES, 128, KT * D)
              .astype(bf16))
    return _chunk_major(xts, B), _chunk_major(wts, D)


def _ensure_trace_shim():
    """If the environment requests NTFF tracing (BASS_TRACE=1) but this
    container's antenv lacks axon_hooks, provide it from trn_boot's ctypes
    implementation so run_bass_kernel_spmd doesn't crash mid-trace."""
    try:
        import antenv.axon_hooks  # noqa: F401
        return
    except ImportError:
        pass
    try:
        import types

        import antenv
        import trn_agent_boot.trn_boot as tb
        from concourse import bass_utils

        hook = tb._ntff_profile_via_ctypes("/opt/axon/libaxon_pjrt.so")
        mod = types.ModuleType("antenv.axon_hooks")
        mod.get_axon_ntff_profile_hook = lambda: hook
        mod.set_axon_ntff_profile_hook = lambda h: None
        antenv.axon_hooks = mod
        sys.modules["antenv.axon_hooks"] = mod
        if not getattr(bass_utils.upload_artifacts, "_patched", False):
            bass_utils.upload_artifacts = lambda tmpdir: tmpdir
            bass_utils.upload_artifacts._patched = True
    except Exception:
        # tracing unavailable -> disable rather than crash the run
        os.environ["BASS_NEVER_TRACE"] = "1"


def kernel(x, weight, isLastLayer=None):
    global LAST_RESULTS
    _ensure_trace_shim()
    from concourse.bass_utils import run_bass_kernel_spmd

    x = np.asarray(x, dtype=np.float32)
    weight = np.asarray(weight, dtype=np.float32)

    xts, wts = _shard_inputs(x, weight)
    in_maps = [{"xt": np.ascontiguousarray(xts[i]),
                "wt": np.ascontiguousarray(wts[i])} for i in range(N_CORES)]

    nc = _get_nc()
    res = run_bass_kernel_spmd(nc, in_maps, core_ids=list(range(N_CORES)))
    LAST_RESULTS = res

    s = np.zeros((B, D), dtype=np.float32)
    for core_out in res.results:
        s += np.asarray(core_out["out"]).astype(np.float32)
    norm = np.sqrt((s.astype(np.float64) ** 2).sum(axis=-1, keepdims=True)).astype(np.float32)
    scale = norm ** 2 / (1.0 + norm ** 2) / (norm + 1e-8)
    return (scale * s)[:, None, :].astype(np.float32)
